# revision 15
# baseline (speedup 1.0000x reference)
"""Trainium2 Bass kernel for IR-Net style binarized 3x3 conv + BN + Hardtanh.

Reference computation:
  bw = sign(standardize(weight)) * sw   (sw = per-cout power-of-2 scale)
  ba = sign(x)
  y  = clip(conv3x3(ba, bw) * bn_scale + bn_bias, -1, 1)

Both matmul operands are exactly +-1, which is exactly representable in
fp8e4m3, so the conv runs as fp8 DoubleRow matmuls on the TensorEngine
with zero numerical error (fp32 PSUM accumulation of integers <= 2304).
All binarization is host-side prep: x ships as fp8 +-1 sign planes that
are already zero-padded and cin-chunk-interleaved, so activations DMA
straight into their SBUF matmul layout — no on-device binarize, border
memsets, or staging.  sw and the BN affine fold into one per-channel
scale/bias applied in the epilogue on VectorE.

Distribution: pure data parallel, 32 images -> 4 per NeuronCore, full
weights replicated, no collectives.

Layout: per-image zero-padded activation planes in SBUF, fp8, with the
two cin-128-chunks byte-interleaved as the DoubleRow k-subtile dim.
Rows are 57 wide (56 data + 1 shared zero column: col 0 is row r's left
pad AND row r-1's right pad), so each of the 9 conv taps is a contiguous
shifted window of the flattened plane and only 1 of every 57 output
columns is garbage.  The conv is 9 accumulated DoubleRow matmuls
([128,2,128] @ [128,2,456], K=256) per 8-row output tile.

The matmul stream runs at the DoubleRow issue-rate ceiling (1 column/
cycle, LDWEIGHTS fully pipelined), so the stream itself is at the fp8
peak; everything here is about the edges:

Startup: PE busy begins at body entry (~7.7us) with warmup matmuls on a
zeroed scratch tile; the HAM clock gate un-throttles after ~5.4us of
UNINTERRUPTED busy (~13.1us — any gap or data stall resets the ramp).
DMA completion carries ~2.2us fixed latency plus large per-DMA
overheads, so the critical set ships as few LARGE pieces: weight taps
0-6 lead the scalar ring, img0 rows 0-10 lead the sync ring — both land
~10.7us.  Warmup is sized to hand off gaplessly to the real stream at
~11.4us, whose first ~5 matmuls run at the throttled clock until the
gate flips — banking ~1us versus warming all the way to the flip.
Later row bands, the co=1 weights, and the bulk images go on the SWDGE
ring (fast, but its completion semaphores lag by a few us — fine for
far-out consumers), gated on warmup progress so their HBM traffic
doesn't contend with the critical pieces.

Tail: output stores alternate between the two HWDGE rings, and the final
row-block of the last (img,co) is split into four 2-row PSUM groups
(small-N matmuls still issue at N+7 cycles — LDWEIGHTS never exposes) so
the last epilogue+store after the final matmul is ~1/4 size and the
post-stream drain the postamble waits on is short.
"""

import numpy as np

import concourse.bass as bass
import concourse.bacc as bacc
import concourse.mybir as mybir
import concourse.tile as tile
from concourse.bass_utils import run_bass_kernel_spmd

B, CIN, COUT, H, W = 32, 256, 256, 56, 56
NCORES = 8
BPC = B // NCORES            # images per core
HP = H + 2                   # padded rows
RW = W + 1                   # row width: 56 data + 1 shared zero col
IMG = HP * RW                # 3306
GUARD = 64                   # front zero guard (shifted windows stay in bounds)
XT = 3376                    # GUARD + IMG + tail guard(6); %16==0 for DoubleRow
RB = 8                       # output rows per tile
NBLK = H // RB               # 7
NCI = CIN // 128             # 2 cin chunks = DoubleRow k-subtiles
NCO = COUT // 128            # 2 cout chunks
KTAPS = 9
BN_EPS = 1e-5

# img0 band split points (tile elem index)
S1 = GUARD + 11 * RW         # rows 0-10 end (first block's reach)
S3 = S1 + 16 * RW            # rows 11-26 end

NWARM = 13                   # warmup matmuls (throttled-clock PE busy bridge)
WN = 300                     # warmup matmul free dim (~250ns each throttled)
NPOST = 20                   # post-stream dummies: hold K=8/8 through the exit

F32 = mybir.dt.float32
BF16 = mybir.dt.bfloat16
FP8 = mybir.dt.float8e4
FP8NP = mybir.dt.np(FP8)

_CACHE: dict = {}


def _build_nc() -> bass.Bass:
    nc = bacc.Bacc("TRN2", target_bir_lowering=False, debug=False, num_devices=NCORES)
    xin8 = nc.declare_dram_parameter("xin8", [BPC, 128, XT * NCI], FP8, isOutput=False)
    wts = nc.declare_dram_parameter(
        "wts", [128, KTAPS * NCO * NCI * 128], FP8, isOutput=False
    )
    sb = nc.declare_dram_parameter("sb", [128, 2 * NCO], F32, isOutput=False)
    yout = nc.declare_dram_parameter("yout", [BPC, COUT, H, W], BF16, isOutput=True)

    with tile.TileContext(nc) as tc:
        with (
            tc.tile_pool(name="const", bufs=1) as cpool,
            tc.tile_pool(name="psum", bufs=8, space=bass.MemorySpace.PSUM) as ppool,
            tc.tile_pool(name="ot", bufs=8) as otpool,
            tc.tile_pool(name="oc", bufs=12) as ocpool,
        ):
            # weights: [p, (co, k, j, m)]
            w_sb = cpool.tile([128, KTAPS * NCO * NCI * 128], FP8, tag="w")
            sb_sb = cpool.tile([128, 2 * NCO], F32, tag="sb")
            WTAP = NCI * 128          # 256 B per tap per partition
            WCO = KTAPS * WTAP        # one cout chunk
            w4 = w_sb.rearrange("p (co k j m) -> p k co j m", k=KTAPS, co=NCO, j=NCI)

            # Scratch for PE warmup operands.  Zeroed by two gpsimd memsets
            # (gpsimd reaches the kernel body first): the stationary 128
            # cols first so the first LDWEIGHTS waits only ~100ns of
            # memset, the moving remainder second.
            wz = cpool.tile([128, WN], FP8, tag="wz")
            nc.gpsimd.memset(wz[:, 0:128], 0.0)
            nc.gpsimd.memset(wz[:, 128:WN], 0.0)

            # Padded binarized activation planes, one tile per image;
            # entirely DMA-written (borders ship as zeros from the host).
            xp = {}
            for img in range(BPC):
                t = cpool.tile([128, XT, NCI], FP8, tag=f"xp{img}", name=f"xp{img}")
                xp[img] = t

            def ld_piece(img, a, b, eng):
                return eng.dma_start(
                    xp[img][:, a:b, :], xin8[img, :, a * NCI : b * NCI]
                )

            def ld_taps(k0, k1, eng, co=0):
                return eng.dma_start(
                    w_sb[:, co * WCO + k0 * WTAP : co * WCO + k1 * WTAP],
                    wts[:, co * WCO + k0 * WTAP : co * WCO + k1 * WTAP],
                )

            # PE warmup: dummy matmuls on the zeroed scratch tile (normal
            # fp8 mode, N=400 -> ~333ns each at the throttled clock).  PE
            # busy starts at body entry; the HAM un-throttle fires ~5.4us
            # of continuous busy later (~13.1us).  Warmup is sized so the
            # real stream starts gaplessly at ~11.4us — data is already
            # there — running its first ~5 matmuls at the throttled clock
            # before the gate flips, which banks ~1us versus warming all
            # the way to the flip.  Any PE idle gap here delays the
            # un-throttle, so the handoff must stay seamless.
            wm_ps = ppool.tile([128, WN], F32, tag="ps")
            wms = []
            for _ in range(NWARM):
                wms.append(nc.tensor.matmul(
                    wm_ps[:],
                    wz[:, 0:128],
                    wz[:, 0:WN],
                    start=True,
                    stop=True,
                ))

            # Startup DMAs.  DMA completion has ~2.2us fixed latency and
            # large per-DMA overhead (descriptor ~0.7us of queue-engine
            # time, ring service per piece), so the critical set ships as
            # FEW, LARGE pieces: weights taps 0-6 lead the scalar ring
            # (first LDWEIGHTS gate), img0 rows 0-10 lead the sync ring
            # (first matmul gate); both land ~10.7us < stream start.
            sq_chain = [
                ld_piece(0, 0, S1, nc.sync),          # img0 rows 0-10
                ld_taps(4, 7, nc.sync),               # taps 4-6
            ]
            sc_chain = [
                ld_taps(0, 4, nc.scalar),             # taps 0-3
                nc.scalar.dma_start(sb_sb[:], sb[:]),  # bn scale/bias
                ld_taps(7, 9, nc.scalar),             # taps 7-8
            ]
            gq_chain = []

            def gate_dma(dma, trigger):
                # real semaphore gate on an early trigger (so the transfer
                # starts promptly) plus a schedule-order-only edge after the
                # first real matmul (keeps the piece from being front-loaded
                # ahead of the critical startup set)
                tile.add_dep_helper(dma.ins, trigger.ins, sync=True,
                                    reason="JIT DMA trigger")
                tile.add_dep_helper(dma.ins, mm0.ins, sync=False,
                                    reason="keep behind critical startup")

            mm0 = None
            for img in range(BPC):
                for co in range(NCO):
                    if img == 0 and co == 1:
                        # co=1 weights + bulk images on the SWDGE ring: fast
                        # transfers but completion semaphores lag a few us —
                        # fine, their consumers are far out.  Gated on
                        # warmup / early-stream progress to stagger HBM
                        # traffic away from the critical startup pieces.
                        wc1 = ld_taps(0, KTAPS, nc.gpsimd, co=1)
                        gate_dma(wc1, wms[2])
                        gq_chain.append(wc1)
                        for im2, trig in ((1, wms[4]), (2, wms[6]),
                                          (3, mm0)):
                            dma = ld_piece(im2, 0, XT, nc.gpsimd)
                            gate_dma(dma, trig)
                            gq_chain.append(dma)
                    s_ap = sb_sb[:, co : co + 1]
                    b_ap = sb_sb[:, NCO + co : NCO + co + 1]
                    # (start padded row, rows, out queue) per output tile.
                    # outputs alternate between the two HWDGE rings (the
                    # gpsimd SWDGE ring's completions lag by several us,
                    # which would stretch the final drain the postamble
                    # waits on)
                    oqs = [nc.sync, nc.scalar]
                    blocks = [
                        (1 + b * RB, RB, oqs[b % 2]) for b in range(NBLK)
                    ]
                    if img == BPC - 1 and co == NCO - 1:
                        # final row-block split 4+4 across both rings: the
                        # epilogue+store after the very last matmul halves,
                        # and the two drains overlap.  No finer — each
                        # store costs ~1.6-2us of ring-packet time however
                        # small, and the drain must finish inside the
                        # ~6.4us semaphore-reset postamble that runs
                        # concurrently after the last engine instruction.
                        blocks = blocks[:-1] + [
                            (49, 4, nc.sync),
                            (53, 4, nc.scalar),
                        ]
                    for bi, (y0p, rb, oq) in enumerate(blocks):
                        if img == 0 and co == 0 and bi == 1:
                            # img0 rows 11-57 on SWDGE (fast transfers,
                            # laggy completion sems — consumers are >2us
                            # out).  Must be EMITTED before the consuming
                            # blocks so Tile sees write-before-read.
                            for (a, b2), trig in (((S1, S3), wms[0]),
                                                  ((S3, XT), wms[1])):
                                dma = ld_piece(0, a, b2, nc.gpsimd)
                                gate_dma(dma, trig)
                                gq_chain.append(dma)
                        nt = rb * RW
                        ps = ppool.tile([128, nt], F32, tag="ps")
                        for k in range(KTAPS):
                            ky, kx = divmod(k, 3)
                            s0 = GUARD + (y0p + ky - 1) * RW + kx
                            rhs = xp[img][:, s0 : s0 + nt, :].rearrange(
                                "p x j -> p j x"
                            )
                            mm = nc.tensor.matmul(
                                ps[:],
                                w4[:, k, co],
                                rhs,
                                start=(k == 0),
                                stop=(k == KTAPS - 1),
                                perf_mode=mybir.MatmulPerfMode.DoubleRow,
                            )
                            if mm0 is None:
                                mm0 = mm
                        ot = otpool.tile([128, nt], F32, tag="ot")
                        nc.vector.tensor_scalar(
                            ot[:],
                            ps[:],
                            s_ap,
                            b_ap,
                            op0=mybir.AluOpType.mult,
                            op1=mybir.AluOpType.add,
                        )
                        # clip + compact away the garbage col per row, so
                        # both sides of the output DMA are fully contiguous.
                        # bf16 output: halves store traffic; quantization is
                        # ~2^-9 relative, far inside the accuracy budget.
                        oc = ocpool.tile([128, rb * W], BF16, tag="oc")
                        nc.vector.tensor_scalar(
                            oc[:],
                            ot.rearrange("p (r c) -> p r c", c=RW)[:, :, 0:W],
                            -1.0,
                            1.0,
                            op0=mybir.AluOpType.max,
                            op1=mybir.AluOpType.min,
                        )
                        # flat dest AP: rows of one channel are contiguous
                        # in DRAM, so this coalesces each partition's store
                        # into one rb*56-elem run instead of per-row pieces
                        oq.dma_start(
                            yout[
                                img, co * 128 : (co + 1) * 128, y0p - 1 : y0p - 1 + rb, :
                            ].rearrange("p r x -> p (r x)"),
                            oc[:],
                        )
            # Post-stream dummies: keep PE busy ~3us past the last real
            # matmul so the HAM clock gate stays at 8/8 while the exit
            # machinery (output drain, semaphore-reset postamble) runs.
            ps_post = ppool.tile([128, WN], F32, tag="ps")
            for _ in range(NPOST):
                nc.tensor.matmul(
                    ps_post[:],
                    wz[:, 0:128],
                    wz[:, 0:WN],
                    start=True,
                    stop=True,
                )

            # pin issue order per ring (ring packet order = issue order)
            for ch in (sc_chain, sq_chain, gq_chain):
                for a, b in zip(ch, ch[1:]):
                    tile.add_dep_helper(
                        b.ins, a.ins, sync=False, reason="startup DMA issue order"
                    )
    nc.finalize()
    return nc


def get_nc() -> bass.Bass:
    if "nc" not in _CACHE:
        _CACHE["nc"] = _build_nc()
    return _CACHE["nc"]


def _host_prep(weight, gamma, beta, running_mean, running_var):
    """Binarize standardized weights, fold sw + BN into scale/bias."""
    wf = weight.reshape(COUT, -1).astype(np.float64)
    n = wf.shape[1]
    mean = wf.mean(axis=1, keepdims=True)
    d = wf - mean
    sgn = np.where(d >= 0, 1.0, -1.0)
    std = np.sqrt((d * d).sum(axis=1, keepdims=True) / (n - 1))
    bw = d / std
    sw = np.exp2(np.round(np.log2(np.abs(bw).mean(axis=1))))  # [COUT]
    inv = gamma.astype(np.float64) / np.sqrt(running_var.astype(np.float64) + BN_EPS)
    scale = (sw * inv).astype(np.float32)
    bias = (beta.astype(np.float64) - running_mean.astype(np.float64) * inv).astype(
        np.float32
    )

    # wts[p, (co, k, j, m)] = sgn[co*128+m, (j*128+p)*9 + k]
    w6 = sgn.reshape(NCO, 128, NCI, 128, KTAPS)  # [co, m, j, p, k]
    wts = (
        np.ascontiguousarray(np.transpose(w6, (3, 0, 4, 2, 1)))  # p co k j m
        .reshape(128, KTAPS * NCO * NCI * 128)
        .astype(FP8NP)
    )
    # sb[m, co] = scale chunk, sb[m, NCO+co] = bias chunk
    sbarr = np.concatenate(
        [scale.reshape(NCO, 128).T, bias.reshape(NCO, 128).T], axis=1
    ).astype(np.float32)
    sbarr = np.ascontiguousarray(sbarr)
    return wts, sbarr


def _host_signs(x):
    """fp8 +-1 sign planes, zero-padded 58x57 rows, cin-chunk interleaved.

    out[b, p, t, j] = fp8(sign(x[b, j*128+p, r-1, c-1])) at t = GUARD+r*57+c
    for the interior, 0 elsewhere (pads/guards), matching torch.sign
    (sign(0) = 0).
    """
    xv = x.reshape(B, NCI, 128, H, W)
    xs = ((xv < 0).astype(np.uint8) * 0x80) | ((xv != 0).astype(np.uint8) * 0x38)
    out = np.zeros((B, 128, XT, NCI), np.uint8)
    interior = out[:, :, GUARD : GUARD + IMG, :].reshape(B, 128, HP, RW, NCI)
    interior[:, :, 1 : H + 1, 1 : W + 1, :] = xs.transpose(0, 2, 3, 4, 1)
    return out.reshape(B, 128, XT * NCI).view(FP8NP)


def run(x, weight, gamma, beta, running_mean, running_var, trace=False, **tkw):
    x = np.asarray(x, dtype=np.float32)
    wts, sbarr = _host_prep(
        np.asarray(weight, dtype=np.float32),
        np.asarray(gamma, dtype=np.float32),
        np.asarray(beta, dtype=np.float32),
        np.asarray(running_mean, dtype=np.float32),
        np.asarray(running_var, dtype=np.float32),
    )
    x8 = _host_signs(x)
    in_maps = [
        {
            "xin8": x8[c * BPC : (c + 1) * BPC],
            "wts": wts,
            "sb": sbarr,
        }
        for c in range(NCORES)
    ]
    nc = get_nc()
    res = run_bass_kernel_spmd(nc, in_maps, list(range(NCORES)), trace=trace, **tkw)
    y = np.concatenate([r["yout"] for r in res.results], axis=0)
    return y.astype(np.float32, copy=False), res


def kernel(x, weight, gamma, beta, running_mean, running_var):
    y, _ = run(x, weight, gamma, beta, running_mean, running_var)
    return y


# revision 19
# speedup vs baseline: 1.0881x; 1.0881x over previous
"""Trainium2 Bass kernel for IR-Net style binarized 3x3 conv + BN + Hardtanh.

Reference computation:
  bw = sign(standardize(weight)) * sw   (sw = per-cout power-of-2 scale)
  ba = sign(x)
  y  = clip(conv3x3(ba, bw) * bn_scale + bn_bias, -1, 1)

Both matmul operands are exactly +-1, which is exactly representable in
fp8e4m3, so the conv runs as fp8 DoubleRow matmuls on the TensorEngine
with zero numerical error (fp32 PSUM accumulation of integers <= 2304).
All binarization is host-side prep: x ships as fp8 +-1 sign planes that
are already zero-padded and cin-chunk-interleaved, so activations DMA
straight into their SBUF matmul layout — no on-device binarize, border
memsets, or staging.  sw and the BN affine fold into one per-channel
scale/bias applied in the epilogue on VectorE.

Distribution: pure data parallel, 32 images -> 4 per NeuronCore, full
weights replicated, no collectives.

Layout: per-image zero-padded activation planes in SBUF, fp8, with the
two cin-128-chunks byte-interleaved as the DoubleRow k-subtile dim.
Rows are 57 wide (56 data + 1 shared zero column: col 0 is row r's left
pad AND row r-1's right pad), so each of the 9 conv taps is a contiguous
shifted window of the flattened plane and only 1 of every 57 output
columns is garbage.  The conv is 9 accumulated DoubleRow matmuls
([128,2,128] @ [128,2,456], K=256) per 8-row output tile.

The matmul stream runs at the DoubleRow issue-rate ceiling (1 column/
cycle, LDWEIGHTS fully pipelined), so the stream itself is at the fp8
peak; everything here is about the edges:

Startup: PE busy begins at body entry (~7.7us) with warmup matmuls on a
zeroed scratch tile; the HAM clock gate un-throttles after ~5.4us of
UNINTERRUPTED busy (~13.1us — any gap or data stall resets the ramp).
DMA completion carries ~2.2us fixed latency plus large per-DMA
overheads, so the critical set ships as few LARGE pieces: weight taps
0-6 lead the scalar ring, img0 rows 0-10 lead the sync ring — both land
~10.7us.  Warmup is sized to hand off gaplessly to the real stream at
~11.4us, whose first ~5 matmuls run at the throttled clock until the
gate flips — banking ~1us versus warming all the way to the flip.
Later row bands, the co=1 weights, and the bulk images go on the SWDGE
ring (fast, but its completion semaphores lag by a few us — fine for
far-out consumers), gated on warmup progress so their HBM traffic
doesn't contend with the critical pieces.

Tail: output stores alternate between the two HWDGE rings, and the final
row-block of the last (img,co) is split into four 2-row PSUM groups
(small-N matmuls still issue at N+7 cycles — LDWEIGHTS never exposes) so
the last epilogue+store after the final matmul is ~1/4 size and the
post-stream drain the postamble waits on is short.
"""

import numpy as np

import concourse.bass as bass
import concourse.bacc as bacc
import concourse.mybir as mybir
import concourse.tile as tile
from concourse.bass_utils import run_bass_kernel_spmd

B, CIN, COUT, H, W = 32, 256, 256, 56, 56
NCORES = 8
BPC = B // NCORES            # images per core
HP = H + 2                   # padded rows
RW = W + 1                   # row width: 56 data + 1 shared zero col
IMG = HP * RW                # 3306
GUARD = 64                   # front zero guard (shifted windows stay in bounds)
XT = 3376                    # GUARD + IMG + tail guard(6); %16==0 for DoubleRow
RB = 8                       # output rows per tile
NBLK = H // RB               # 7
NCI = CIN // 128             # 2 cin chunks = DoubleRow k-subtiles
NCO = COUT // 128            # 2 cout chunks
KTAPS = 9
BN_EPS = 1e-5

# img0 band split points (tile elem index)
S1 = GUARD + 11 * RW         # rows 0-10 end (first block's reach)
S3 = S1 + 16 * RW            # rows 11-26 end

NWARM = 13                   # warmup matmuls (throttled-clock PE busy bridge)
WN = 300                     # warmup matmul free dim (~250ns each throttled)
NPOST = 20                   # post-stream dummies: hold K=8/8 through the exit

F32 = mybir.dt.float32
BF16 = mybir.dt.bfloat16
FP8 = mybir.dt.float8e4
FP8NP = mybir.dt.np(FP8)

_CACHE: dict = {}


def _build_nc() -> bass.Bass:
    nc = bacc.Bacc("TRN2", target_bir_lowering=False, debug=False, num_devices=NCORES)
    xin8 = nc.declare_dram_parameter("xin8", [BPC, 128, XT * NCI], FP8, isOutput=False)
    wts = nc.declare_dram_parameter(
        "wts", [128, KTAPS * NCO * NCI * 128], FP8, isOutput=False
    )
    sb = nc.declare_dram_parameter("sb", [128, 2 * NCO], F32, isOutput=False)
    yout = nc.declare_dram_parameter("yout", [BPC, COUT, H, W], BF16, isOutput=True)

    with tile.TileContext(nc) as tc:
        with (
            tc.tile_pool(name="const", bufs=1) as cpool,
            tc.tile_pool(name="psum", bufs=8, space=bass.MemorySpace.PSUM) as ppool,
            tc.tile_pool(name="ot", bufs=8) as otpool,
            tc.tile_pool(name="oc", bufs=12) as ocpool,
        ):
            # weights: [p, (co, k, j, m)]
            w_sb = cpool.tile([128, KTAPS * NCO * NCI * 128], FP8, tag="w")
            sb_sb = cpool.tile([128, 2 * NCO], F32, tag="sb")
            WTAP = NCI * 128          # 256 B per tap per partition
            WCO = KTAPS * WTAP        # one cout chunk
            w4 = w_sb.rearrange("p (co k j m) -> p k co j m", k=KTAPS, co=NCO, j=NCI)

            # Scratch for PE warmup operands.  Zeroed by two gpsimd memsets
            # (gpsimd reaches the kernel body first): the stationary 128
            # cols first so the first LDWEIGHTS waits only ~100ns of
            # memset, the moving remainder second.  Shaped [128, WN, 2] so
            # warmups can run as DoubleRow matmuls with operand APs
            # structurally identical to the real stream's — same perf mode
            # and ~2x the MACs/cycle of normal mode, so the HAM utilization
            # ramp sees the same load it will need to sustain.
            wzw = cpool.tile([128, NCI * 128], FP8, tag="wzw")
            wz = cpool.tile([128, WN, 2], FP8, tag="wz")
            nc.gpsimd.memset(wzw[:], 0.0)
            nc.gpsimd.memset(wz[:], 0.0)
            wz_l = wzw.rearrange("p (j m) -> p j m", j=NCI)
            wz_r = wz[:, 0:WN, :].rearrange("p x j -> p j x")

            # Padded binarized activation planes, one tile per image;
            # entirely DMA-written (borders ship as zeros from the host).
            xp = {}
            for img in range(BPC):
                t = cpool.tile([128, XT, NCI], FP8, tag=f"xp{img}", name=f"xp{img}")
                xp[img] = t

            def ld_piece(img, a, b, eng):
                return eng.dma_start(
                    xp[img][:, a:b, :], xin8[img, :, a * NCI : b * NCI]
                )

            def ld_taps(k0, k1, eng, co=0):
                return eng.dma_start(
                    w_sb[:, co * WCO + k0 * WTAP : co * WCO + k1 * WTAP],
                    wts[:, co * WCO + k0 * WTAP : co * WCO + k1 * WTAP],
                )

            # PE warmup: dummy matmuls on the zeroed scratch tile (normal
            # fp8 mode, N=400 -> ~333ns each at the throttled clock).  PE
            # busy starts at body entry; the HAM un-throttle fires ~5.4us
            # of continuous busy later (~13.1us).  Warmup is sized so the
            # real stream starts gaplessly at ~11.4us — data is already
            # there — running its first ~5 matmuls at the throttled clock
            # before the gate flips, which banks ~1us versus warming all
            # the way to the flip.  Any PE idle gap here delays the
            # un-throttle, so the handoff must stay seamless.
            wm_ps = ppool.tile([128, WN], F32, tag="ps")
            wms = []
            for _ in range(NWARM):
                wms.append(nc.tensor.matmul(
                    wm_ps[:],
                    wz_l,
                    wz_r,
                    start=True,
                    stop=True,
                    perf_mode=mybir.MatmulPerfMode.DoubleRow,
                ))

            # Startup DMAs.  DMA completion has ~2.2us fixed latency and
            # large per-DMA overhead (descriptor ~0.7us of queue-engine
            # time, ring service per piece), so the critical set ships as
            # FEW, LARGE pieces: weights taps 0-6 lead the scalar ring
            # (first LDWEIGHTS gate), img0 rows 0-10 lead the sync ring
            # (first matmul gate); both land ~10.7us < stream start.
            sq_chain = [
                ld_piece(0, 0, S1, nc.sync),          # img0 rows 0-10
                ld_taps(4, 7, nc.sync),               # taps 4-6
            ]
            sc_chain = [
                ld_taps(0, 4, nc.scalar),             # taps 0-3
                nc.scalar.dma_start(sb_sb[:], sb[:]),  # bn scale/bias
                ld_taps(7, 9, nc.scalar),             # taps 7-8
            ]
            gq_chain = []

            def gate_dma(dma, trigger):
                # real semaphore gate on an early trigger (so the transfer
                # starts promptly) plus a schedule-order-only edge after the
                # first real matmul (keeps the piece from being front-loaded
                # ahead of the critical startup set)
                tile.add_dep_helper(dma.ins, trigger.ins, sync=True,
                                    reason="JIT DMA trigger")
                tile.add_dep_helper(dma.ins, mm0.ins, sync=False,
                                    reason="keep behind critical startup")

            mm0 = None
            for img in range(BPC):
                for co in range(NCO):
                    if img == 0 and co == 1:
                        # co=1 weights + bulk images on the SWDGE ring: fast
                        # transfers but completion semaphores lag a few us —
                        # fine, their consumers are far out.  Gated on
                        # warmup / early-stream progress to stagger HBM
                        # traffic away from the critical startup pieces.
                        wc1 = ld_taps(0, KTAPS, nc.gpsimd, co=1)
                        gate_dma(wc1, wms[2])
                        gq_chain.append(wc1)
                        for im2, trig in ((1, wms[4]), (2, wms[6]),
                                          (3, mm0)):
                            dma = ld_piece(im2, 0, XT, nc.gpsimd)
                            gate_dma(dma, trig)
                            gq_chain.append(dma)
                    s_ap = sb_sb[:, co : co + 1]
                    b_ap = sb_sb[:, NCO + co : NCO + co + 1]
                    # (start padded row, rows, out queue) per output tile.
                    # outputs alternate between the two HWDGE rings (the
                    # gpsimd SWDGE ring's completions lag by several us,
                    # which would stretch the final drain the postamble
                    # waits on)
                    oqs = [nc.sync, nc.scalar]
                    blocks = [
                        (1 + b * RB, RB, oqs[b % 2]) for b in range(NBLK)
                    ]
                    if img == BPC - 1 and co == NCO - 1:
                        # final row-block split 4+4 across both rings: the
                        # epilogue+store after the very last matmul halves,
                        # and the two drains overlap.  No finer — each
                        # store costs ~1.6-2us of ring-packet time however
                        # small, and the drain must finish inside the
                        # ~6.4us semaphore-reset postamble that runs
                        # concurrently after the last engine instruction.
                        blocks = blocks[:-1] + [
                            (49, 4, nc.sync),
                            (53, 4, nc.scalar),
                        ]
                    for bi, (y0p, rb, oq) in enumerate(blocks):
                        if img == 0 and co == 0 and bi == 1:
                            # img0 rows 11-57 on SWDGE (fast transfers,
                            # laggy completion sems — consumers are >2us
                            # out).  Must be EMITTED before the consuming
                            # blocks so Tile sees write-before-read.
                            for (a, b2), trig in (((S1, S3), wms[0]),
                                                  ((S3, XT), wms[1])):
                                dma = ld_piece(0, a, b2, nc.gpsimd)
                                gate_dma(dma, trig)
                                gq_chain.append(dma)
                        nt = rb * RW
                        ps = ppool.tile([128, nt], F32, tag="ps")
                        for k in range(KTAPS):
                            ky, kx = divmod(k, 3)
                            s0 = GUARD + (y0p + ky - 1) * RW + kx
                            rhs = xp[img][:, s0 : s0 + nt, :].rearrange(
                                "p x j -> p j x"
                            )
                            mm = nc.tensor.matmul(
                                ps[:],
                                w4[:, k, co],
                                rhs,
                                start=(k == 0),
                                stop=(k == KTAPS - 1),
                                perf_mode=mybir.MatmulPerfMode.DoubleRow,
                            )
                            if mm0 is None:
                                mm0 = mm
                        ot = otpool.tile([128, nt], F32, tag="ot")
                        nc.vector.tensor_scalar(
                            ot[:],
                            ps[:],
                            s_ap,
                            b_ap,
                            op0=mybir.AluOpType.mult,
                            op1=mybir.AluOpType.add,
                        )
                        # clip + compact away the garbage col per row, so
                        # both sides of the output DMA are fully contiguous.
                        # bf16 output: halves store traffic; quantization is
                        # ~2^-9 relative, far inside the accuracy budget.
                        oc = ocpool.tile([128, rb * W], BF16, tag="oc")
                        nc.vector.tensor_scalar(
                            oc[:],
                            ot.rearrange("p (r c) -> p r c", c=RW)[:, :, 0:W],
                            -1.0,
                            1.0,
                            op0=mybir.AluOpType.max,
                            op1=mybir.AluOpType.min,
                        )
                        # flat dest AP: rows of one channel are contiguous
                        # in DRAM, so this coalesces each partition's store
                        # into one rb*56-elem run instead of per-row pieces
                        oq.dma_start(
                            yout[
                                img, co * 128 : (co + 1) * 128, y0p - 1 : y0p - 1 + rb, :
                            ].rearrange("p r x -> p (r x)"),
                            oc[:],
                        )
            # Post-stream dummies: keep PE busy ~3us past the last real
            # matmul so the HAM clock gate stays at 8/8 while the exit
            # machinery (output drain, semaphore-reset postamble) runs.
            ps_post = ppool.tile([128, WN], F32, tag="ps")
            for _ in range(NPOST):
                nc.tensor.matmul(
                    ps_post[:],
                    wz_l,
                    wz_r,
                    start=True,
                    stop=True,
                    perf_mode=mybir.MatmulPerfMode.DoubleRow,
                )

            # pin issue order per ring (ring packet order = issue order)
            for ch in (sc_chain, sq_chain, gq_chain):
                for a, b in zip(ch, ch[1:]):
                    tile.add_dep_helper(
                        b.ins, a.ins, sync=False, reason="startup DMA issue order"
                    )
    nc.finalize()
    return nc


def get_nc() -> bass.Bass:
    if "nc" not in _CACHE:
        _CACHE["nc"] = _build_nc()
    return _CACHE["nc"]


def _host_prep(weight, gamma, beta, running_mean, running_var):
    """Binarize standardized weights, fold sw + BN into scale/bias."""
    wf = weight.reshape(COUT, -1).astype(np.float64)
    n = wf.shape[1]
    mean = wf.mean(axis=1, keepdims=True)
    d = wf - mean
    sgn = np.where(d >= 0, 1.0, -1.0)
    std = np.sqrt((d * d).sum(axis=1, keepdims=True) / (n - 1))
    bw = d / std
    sw = np.exp2(np.round(np.log2(np.abs(bw).mean(axis=1))))  # [COUT]
    inv = gamma.astype(np.float64) / np.sqrt(running_var.astype(np.float64) + BN_EPS)
    scale = (sw * inv).astype(np.float32)
    bias = (beta.astype(np.float64) - running_mean.astype(np.float64) * inv).astype(
        np.float32
    )

    # wts[p, (co, k, j, m)] = sgn[co*128+m, (j*128+p)*9 + k]
    w6 = sgn.reshape(NCO, 128, NCI, 128, KTAPS)  # [co, m, j, p, k]
    wts = (
        np.ascontiguousarray(np.transpose(w6, (3, 0, 4, 2, 1)))  # p co k j m
        .reshape(128, KTAPS * NCO * NCI * 128)
        .astype(FP8NP)
    )
    # sb[m, co] = scale chunk, sb[m, NCO+co] = bias chunk
    sbarr = np.concatenate(
        [scale.reshape(NCO, 128).T, bias.reshape(NCO, 128).T], axis=1
    ).astype(np.float32)
    sbarr = np.ascontiguousarray(sbarr)
    return wts, sbarr


def _host_signs(x):
    """fp8 +-1 sign planes, zero-padded 58x57 rows, cin-chunk interleaved.

    out[b, p, t, j] = fp8(sign(x[b, j*128+p, r-1, c-1])) at t = GUARD+r*57+c
    for the interior, 0 elsewhere (pads/guards), matching torch.sign
    (sign(0) = 0).
    """
    xv = x.reshape(B, NCI, 128, H, W)
    xs = ((xv < 0).astype(np.uint8) * 0x80) | ((xv != 0).astype(np.uint8) * 0x38)
    out = np.zeros((B, 128, XT, NCI), np.uint8)
    interior = out[:, :, GUARD : GUARD + IMG, :].reshape(B, 128, HP, RW, NCI)
    interior[:, :, 1 : H + 1, 1 : W + 1, :] = xs.transpose(0, 2, 3, 4, 1)
    return out.reshape(B, 128, XT * NCI).view(FP8NP)


def run(x, weight, gamma, beta, running_mean, running_var, trace=False, **tkw):
    x = np.asarray(x, dtype=np.float32)
    wts, sbarr = _host_prep(
        np.asarray(weight, dtype=np.float32),
        np.asarray(gamma, dtype=np.float32),
        np.asarray(beta, dtype=np.float32),
        np.asarray(running_mean, dtype=np.float32),
        np.asarray(running_var, dtype=np.float32),
    )
    x8 = _host_signs(x)
    in_maps = [
        {
            "xin8": x8[c * BPC : (c + 1) * BPC],
            "wts": wts,
            "sb": sbarr,
        }
        for c in range(NCORES)
    ]
    nc = get_nc()
    res = run_bass_kernel_spmd(nc, in_maps, list(range(NCORES)), trace=trace, **tkw)
    y = np.concatenate([r["yout"] for r in res.results], axis=0)
    return y.astype(np.float32, copy=False), res


def kernel(x, weight, gamma, beta, running_mean, running_var):
    y, _ = run(x, weight, gamma, beta, running_mean, running_var)
    return y


# revision 22
# speedup vs baseline: 1.0921x; 1.0037x over previous
"""Trainium2 Bass kernel for IR-Net style binarized 3x3 conv + BN + Hardtanh.

Reference computation:
  bw = sign(standardize(weight)) * sw   (sw = per-cout power-of-2 scale)
  ba = sign(x)
  y  = clip(conv3x3(ba, bw) * bn_scale + bn_bias, -1, 1)

Both matmul operands are exactly +-1, which is exactly representable in
fp8e4m3, so the conv runs as fp8 DoubleRow matmuls on the TensorEngine
with zero numerical error (fp32 PSUM accumulation of integers <= 2304).
All binarization is host-side prep: x ships as fp8 +-1 sign planes that
are already zero-padded and cin-chunk-interleaved, so activations DMA
straight into their SBUF matmul layout — no on-device binarize, border
memsets, or staging.  sw and the BN affine fold into one per-channel
scale/bias applied in the epilogue on VectorE.

Distribution: pure data parallel, 32 images -> 4 per NeuronCore, full
weights replicated, no collectives.

Layout: per-image zero-padded activation planes in SBUF, fp8, with the
two cin-128-chunks byte-interleaved as the DoubleRow k-subtile dim.
Rows are 57 wide (56 data + 1 shared zero column: col 0 is row r's left
pad AND row r-1's right pad), so each of the 9 conv taps is a contiguous
shifted window of the flattened plane and only 1 of every 57 output
columns is garbage.  The conv is 9 accumulated DoubleRow matmuls
([128,2,128] @ [128,2,456], K=256) per 8-row output tile.

The matmul stream runs at the DoubleRow issue-rate ceiling (1 column/
cycle, LDWEIGHTS fully pipelined), so the stream itself is at the fp8
peak; everything here is about the edges:

Startup: PE busy begins at body entry (~7.5us) with warmup matmuls on a
zeroed scratch tile (DoubleRow, operand APs structurally identical to
the real stream's); the HAM clock gate un-throttles ~5.0us of
UNINTERRUPTED busy later (~12.5us — any gap or data stall delays it,
measured worse than the throttled-run cost).  DMA completion carries
~1.5-2us fixed latency plus large per-DMA overheads (~0.7us of
queue-engine descriptor time, ~8 ring packets each at ~150-300ns
however small the piece), so the critical set ships as few LARGE
pieces: img0 rows 0-10 + taps 4-6 on the sync ring, taps 0-3 +
scale/bias + taps 7-8 on the scalar ring; the first block's gates land
~10.3us.  Warmup is sized to hand off gaplessly right there, so the
first ~5 real matmuls run at the still-throttled clock until the gate
flips — banking ~1us versus warming all the way to the flip.  Later
row bands, the co=1 weights, and the bulk images go on the SWDGE ring
(fast, but its completion semaphores lag by a few us — fine for
far-out consumers), gated on warmup progress so their HBM traffic
doesn't contend with the critical pieces.

Tail: output stores alternate between the two HWDGE rings, and the final
row-block of the last (img,co) is split 4+4 across both rings (small-N
matmuls still issue at N+7 cycles — LDWEIGHTS never exposes) so the
last epilogue+store after the final matmul is halved and the two drains
overlap.  A few post-stream dummy matmuls hold the clock gate at 8/8
through the exit.  The remaining ~11.3us after the last matmul is the
runtime-inserted NEFF postamble (per-semaphore zeroing chains at a
fixed ~122ns cadence, exit barriers) — invariant to kernel structure.
"""

import numpy as np

import concourse.bass as bass
import concourse.bacc as bacc
import concourse.mybir as mybir
import concourse.tile as tile
from concourse.bass_utils import run_bass_kernel_spmd

B, CIN, COUT, H, W = 32, 256, 256, 56, 56
NCORES = 8
BPC = B // NCORES            # images per core
HP = H + 2                   # padded rows
RW = W + 1                   # row width: 56 data + 1 shared zero col
IMG = HP * RW                # 3306
GUARD = 64                   # front zero guard (shifted windows stay in bounds)
XT = 3376                    # GUARD + IMG + tail guard(6); %16==0 for DoubleRow
RB = 8                       # output rows per tile
NBLK = H // RB               # 7
NCI = CIN // 128             # 2 cin chunks = DoubleRow k-subtiles
NCO = COUT // 128            # 2 cout chunks
KTAPS = 9
BN_EPS = 1e-5

# img0 band split points (tile elem index)
S1 = GUARD + 11 * RW         # rows 0-10 end (first block's reach)
S3 = S1 + 16 * RW            # rows 11-26 end

NWARM = 13                   # warmup matmuls (throttled-clock PE busy bridge)
WN = 300                     # warmup matmul free dim (~250ns each throttled)
NPOST = 20                   # post-stream dummies: hold K=8/8 through the exit

F32 = mybir.dt.float32
BF16 = mybir.dt.bfloat16
FP8 = mybir.dt.float8e4
FP8NP = mybir.dt.np(FP8)

_CACHE: dict = {}


def _build_nc() -> bass.Bass:
    nc = bacc.Bacc("TRN2", target_bir_lowering=False, debug=False, num_devices=NCORES)
    xin8 = nc.declare_dram_parameter("xin8", [BPC, 128, XT * NCI], FP8, isOutput=False)
    wts = nc.declare_dram_parameter(
        "wts", [128, KTAPS * NCO * NCI * 128], FP8, isOutput=False
    )
    sb = nc.declare_dram_parameter("sb", [128, 2 * NCO], F32, isOutput=False)
    yout = nc.declare_dram_parameter("yout", [BPC, COUT, H, W], BF16, isOutput=True)

    with tile.TileContext(nc) as tc:
        with (
            tc.tile_pool(name="const", bufs=1) as cpool,
            tc.tile_pool(name="psum", bufs=8, space=bass.MemorySpace.PSUM) as ppool,
            tc.tile_pool(name="ot", bufs=8) as otpool,
            tc.tile_pool(name="oc", bufs=12) as ocpool,
        ):
            # weights: [p, (co, k, j, m)]
            w_sb = cpool.tile([128, KTAPS * NCO * NCI * 128], FP8, tag="w")
            sb_sb = cpool.tile([128, 2 * NCO], F32, tag="sb")
            WTAP = NCI * 128          # 256 B per tap per partition
            WCO = KTAPS * WTAP        # one cout chunk
            w4 = w_sb.rearrange("p (co k j m) -> p k co j m", k=KTAPS, co=NCO, j=NCI)

            # Scratch for PE warmup operands.  Zeroed by two gpsimd memsets
            # (gpsimd reaches the kernel body first): the stationary 128
            # cols first so the first LDWEIGHTS waits only ~100ns of
            # memset, the moving remainder second.  Shaped [128, WN, 2] so
            # warmups can run as DoubleRow matmuls with operand APs
            # structurally identical to the real stream's — same perf mode
            # and ~2x the MACs/cycle of normal mode, so the HAM utilization
            # ramp sees the same load it will need to sustain.
            wzw = cpool.tile([128, NCI * 128], FP8, tag="wzw")
            wz = cpool.tile([128, WN, 2], FP8, tag="wz")
            nc.gpsimd.memset(wzw[:], 0.0)
            nc.gpsimd.memset(wz[:], 0.0)
            wz_l = wzw.rearrange("p (j m) -> p j m", j=NCI)
            wz_r = wz[:, 0:WN, :].rearrange("p x j -> p j x")

            # Padded binarized activation planes, one tile per image;
            # entirely DMA-written (borders ship as zeros from the host).
            xp = {}
            for img in range(BPC):
                t = cpool.tile([128, XT, NCI], FP8, tag=f"xp{img}", name=f"xp{img}")
                xp[img] = t

            def ld_piece(img, a, b, eng):
                return eng.dma_start(
                    xp[img][:, a:b, :], xin8[img, :, a * NCI : b * NCI]
                )

            def ld_taps(k0, k1, eng, co=0):
                return eng.dma_start(
                    w_sb[:, co * WCO + k0 * WTAP : co * WCO + k1 * WTAP],
                    wts[:, co * WCO + k0 * WTAP : co * WCO + k1 * WTAP],
                )

            # PE warmup: dummy DoubleRow matmuls on the zeroed scratch
            # tile (N=300 -> ~250ns each at the throttled clock).  PE busy
            # starts at body entry; the HAM un-throttle fires ~5.0us of
            # continuous busy later.  Warmup is sized so the real stream
            # starts gaplessly right as its data lands (~10.9us), running
            # its first ~5 matmuls at the throttled clock before the gate
            # flips — banking ~1us versus warming all the way to the
            # flip.  Any PE idle gap here delays the un-throttle, so the
            # handoff must stay seamless.
            wm_ps = ppool.tile([128, WN], F32, tag="ps")
            wms = []
            for _ in range(NWARM):
                wms.append(nc.tensor.matmul(
                    wm_ps[:],
                    wz_l,
                    wz_r,
                    start=True,
                    stop=True,
                    perf_mode=mybir.MatmulPerfMode.DoubleRow,
                ))

            # Startup DMAs.  DMA completion has ~2.2us fixed latency and
            # large per-DMA overhead (descriptor ~0.7us of queue-engine
            # time, ring service per piece), so the critical set ships as
            # FEW, LARGE pieces: weights taps 0-6 lead the scalar ring
            # (first LDWEIGHTS gate), img0 rows 0-10 lead the sync ring
            # (first matmul gate); both land ~10.7us < stream start.
            sq_chain = [
                ld_piece(0, 0, S1, nc.sync),          # img0 rows 0-10
                ld_taps(4, 7, nc.sync),               # taps 4-6
            ]
            sc_chain = [
                ld_taps(0, 4, nc.scalar),             # taps 0-3
                nc.scalar.dma_start(sb_sb[:], sb[:]),  # bn scale/bias
                ld_taps(7, 9, nc.scalar),             # taps 7-8
            ]
            gq_chain = []

            def gate_dma(dma, trigger):
                # real semaphore gate on an early trigger (so the transfer
                # starts promptly) plus a schedule-order-only edge after the
                # first real matmul (keeps the piece from being front-loaded
                # ahead of the critical startup set)
                tile.add_dep_helper(dma.ins, trigger.ins, sync=True,
                                    reason="JIT DMA trigger")
                tile.add_dep_helper(dma.ins, mm0.ins, sync=False,
                                    reason="keep behind critical startup")

            mm0 = None
            for img in range(BPC):
                for co in range(NCO):
                    if img == 0 and co == 1:
                        # co=1 weights + bulk images on the SWDGE ring: fast
                        # transfers but completion semaphores lag a few us —
                        # fine, their consumers are far out.  Gated on
                        # warmup / early-stream progress to stagger HBM
                        # traffic away from the critical startup pieces.
                        wc1 = ld_taps(0, KTAPS, nc.gpsimd, co=1)
                        gate_dma(wc1, wms[2])
                        gq_chain.append(wc1)
                        for im2, trig in ((1, wms[4]), (2, wms[6]),
                                          (3, mm0)):
                            dma = ld_piece(im2, 0, XT, nc.gpsimd)
                            gate_dma(dma, trig)
                            gq_chain.append(dma)
                    s_ap = sb_sb[:, co : co + 1]
                    b_ap = sb_sb[:, NCO + co : NCO + co + 1]
                    # (start padded row, rows, out queue) per output tile.
                    # outputs alternate between the two HWDGE rings (the
                    # gpsimd SWDGE ring's completions lag by several us,
                    # which would stretch the final drain the postamble
                    # waits on)
                    oqs = [nc.sync, nc.scalar]
                    blocks = [
                        (1 + b * RB, RB, oqs[b % 2]) for b in range(NBLK)
                    ]
                    if img == BPC - 1 and co == NCO - 1:
                        # final row-block split 4+4 across both rings: the
                        # epilogue+store after the very last matmul halves,
                        # and the two drains overlap.  No finer — each
                        # store costs ~1.6-2us of ring-packet time however
                        # small, and the drain must finish inside the
                        # ~6.4us semaphore-reset postamble that runs
                        # concurrently after the last engine instruction.
                        blocks = blocks[:-1] + [
                            (49, 4, nc.sync),
                            (53, 4, nc.scalar),
                        ]
                    for bi, (y0p, rb, oq) in enumerate(blocks):
                        if img == 0 and co == 0 and bi == 1:
                            # img0 rows 11-57 on SWDGE (fast transfers,
                            # laggy completion sems — consumers are >2us
                            # out).  Must be EMITTED before the consuming
                            # blocks so Tile sees write-before-read.
                            for (a, b2), trig in (((S1, S3), wms[0]),
                                                  ((S3, XT), wms[1])):
                                dma = ld_piece(0, a, b2, nc.gpsimd)
                                gate_dma(dma, trig)
                                gq_chain.append(dma)
                        nt = rb * RW
                        ps = ppool.tile([128, nt], F32, tag="ps")
                        for k in range(KTAPS):
                            ky, kx = divmod(k, 3)
                            s0 = GUARD + (y0p + ky - 1) * RW + kx
                            rhs = xp[img][:, s0 : s0 + nt, :].rearrange(
                                "p x j -> p j x"
                            )
                            mm = nc.tensor.matmul(
                                ps[:],
                                w4[:, k, co],
                                rhs,
                                start=(k == 0),
                                stop=(k == KTAPS - 1),
                                perf_mode=mybir.MatmulPerfMode.DoubleRow,
                            )
                            if mm0 is None:
                                mm0 = mm
                        ot = otpool.tile([128, nt], F32, tag="ot")
                        nc.vector.tensor_scalar(
                            ot[:],
                            ps[:],
                            s_ap,
                            b_ap,
                            op0=mybir.AluOpType.mult,
                            op1=mybir.AluOpType.add,
                        )
                        # clip + compact away the garbage col per row, so
                        # both sides of the output DMA are fully contiguous.
                        # bf16 output: halves store traffic; quantization is
                        # ~2^-9 relative, far inside the accuracy budget.
                        oc = ocpool.tile([128, rb * W], BF16, tag="oc")
                        nc.vector.tensor_scalar(
                            oc[:],
                            ot.rearrange("p (r c) -> p r c", c=RW)[:, :, 0:W],
                            -1.0,
                            1.0,
                            op0=mybir.AluOpType.max,
                            op1=mybir.AluOpType.min,
                        )
                        # flat dest AP: rows of one channel are contiguous
                        # in DRAM, so this coalesces each partition's store
                        # into one rb*56-elem run instead of per-row pieces
                        oq.dma_start(
                            yout[
                                img, co * 128 : (co + 1) * 128, y0p - 1 : y0p - 1 + rb, :
                            ].rearrange("p r x -> p (r x)"),
                            oc[:],
                        )
            # Post-stream dummies: keep PE busy ~3us past the last real
            # matmul so the HAM clock gate stays at 8/8 while the exit
            # machinery (output drain, semaphore-reset postamble) runs.
            ps_post = ppool.tile([128, WN], F32, tag="ps")
            for _ in range(NPOST):
                nc.tensor.matmul(
                    ps_post[:],
                    wz_l,
                    wz_r,
                    start=True,
                    stop=True,
                    perf_mode=mybir.MatmulPerfMode.DoubleRow,
                )

            # pin issue order per ring (ring packet order = issue order)
            for ch in (sc_chain, sq_chain, gq_chain):
                for a, b in zip(ch, ch[1:]):
                    tile.add_dep_helper(
                        b.ins, a.ins, sync=False, reason="startup DMA issue order"
                    )
    nc.finalize()
    return nc


def get_nc() -> bass.Bass:
    if "nc" not in _CACHE:
        _CACHE["nc"] = _build_nc()
    return _CACHE["nc"]


def _host_prep(weight, gamma, beta, running_mean, running_var):
    """Binarize standardized weights, fold sw + BN into scale/bias."""
    wf = weight.reshape(COUT, -1).astype(np.float64)
    n = wf.shape[1]
    mean = wf.mean(axis=1, keepdims=True)
    d = wf - mean
    sgn = np.where(d >= 0, 1.0, -1.0)
    std = np.sqrt((d * d).sum(axis=1, keepdims=True) / (n - 1))
    bw = d / std
    sw = np.exp2(np.round(np.log2(np.abs(bw).mean(axis=1))))  # [COUT]
    inv = gamma.astype(np.float64) / np.sqrt(running_var.astype(np.float64) + BN_EPS)
    scale = (sw * inv).astype(np.float32)
    bias = (beta.astype(np.float64) - running_mean.astype(np.float64) * inv).astype(
        np.float32
    )

    # wts[p, (co, k, j, m)] = sgn[co*128+m, (j*128+p)*9 + k]
    w6 = sgn.reshape(NCO, 128, NCI, 128, KTAPS)  # [co, m, j, p, k]
    wts = (
        np.ascontiguousarray(np.transpose(w6, (3, 0, 4, 2, 1)))  # p co k j m
        .reshape(128, KTAPS * NCO * NCI * 128)
        .astype(FP8NP)
    )
    # sb[m, co] = scale chunk, sb[m, NCO+co] = bias chunk
    sbarr = np.concatenate(
        [scale.reshape(NCO, 128).T, bias.reshape(NCO, 128).T], axis=1
    ).astype(np.float32)
    sbarr = np.ascontiguousarray(sbarr)
    return wts, sbarr


def _host_signs(x):
    """fp8 +-1 sign planes, zero-padded 58x57 rows, cin-chunk interleaved.

    out[b, p, t, j] = fp8(sign(x[b, j*128+p, r-1, c-1])) at t = GUARD+r*57+c
    for the interior, 0 elsewhere (pads/guards), matching torch.sign
    (sign(0) = 0).
    """
    xv = x.reshape(B, NCI, 128, H, W)
    xs = ((xv < 0).astype(np.uint8) * 0x80) | ((xv != 0).astype(np.uint8) * 0x38)
    out = np.zeros((B, 128, XT, NCI), np.uint8)
    interior = out[:, :, GUARD : GUARD + IMG, :].reshape(B, 128, HP, RW, NCI)
    interior[:, :, 1 : H + 1, 1 : W + 1, :] = xs.transpose(0, 2, 3, 4, 1)
    return out.reshape(B, 128, XT * NCI).view(FP8NP)


def run(x, weight, gamma, beta, running_mean, running_var, trace=False, **tkw):
    x = np.asarray(x, dtype=np.float32)
    wts, sbarr = _host_prep(
        np.asarray(weight, dtype=np.float32),
        np.asarray(gamma, dtype=np.float32),
        np.asarray(beta, dtype=np.float32),
        np.asarray(running_mean, dtype=np.float32),
        np.asarray(running_var, dtype=np.float32),
    )
    x8 = _host_signs(x)
    in_maps = [
        {
            "xin8": x8[c * BPC : (c + 1) * BPC],
            "wts": wts,
            "sb": sbarr,
        }
        for c in range(NCORES)
    ]
    nc = get_nc()
    res = run_bass_kernel_spmd(nc, in_maps, list(range(NCORES)), trace=trace, **tkw)
    y = np.concatenate([r["yout"] for r in res.results], axis=0)
    return y.astype(np.float32, copy=False), res


def kernel(x, weight, gamma, beta, running_mean, running_var):
    y, _ = run(x, weight, gamma, beta, running_mean, running_var)
    return y
